# revision 10
# baseline (speedup 1.0000x reference)
"""DCT non-local attention (nn_DCTNLAttention11) Trainium2 kernel.

Data-parallel over batch B=8 across 8 NeuronCores; each core processes one
batch element [C=512, HW=16384].  All constants derived from the DCT basis P
are precomputed on host; the per-core device program is:

  1. xPT = P^T @ x^T            [64, 512]   (128 accumulated matmuls over n)
  2. xP (PE transposes), then W-projections off xP (tiny matmuls):
     WqxP^T/WkxP^T/WvxP^T, WqxP/WkxP, fatt = WkxP^T @ WqxP
  3. Per-n norm columns: QT/KT chunks [128,64] via PT-chunk-stationary
     matmuls; lamq2/lamk2 via ACT Square with accum_out; rq = 1/sqrt.
  4. Pk = P * (1/lamdk) per chunk; A_ext = Pk^T @ [P|1] (accumulated).
  5. M1T/rowv = fatt^T @ [A|s]; lamdv_pre columns; rv/rqv columns -> flat rows.
  6. Per 512-col chunk: G = M1 @ PT, T = G*rqv + S*rv (rank-1 matmuls + DVE),
     out_tile = [gamma*WvxP^T; gamma*bias]^T @ [T; 1] + x   (residual in bf16)
"""

import numpy as np
import ml_dtypes
from contextlib import ExitStack

import concourse.bass as bass
import concourse.bacc as bacc
import concourse.tile as tile
from concourse import mybir
from concourse.bass_utils import run_bass_kernel_spmd

F32 = mybir.dt.float32
BF16 = mybir.dt.bfloat16
AF = mybir.ActivationFunctionType
BF16_NP = ml_dtypes.bfloat16

B, C, H, W = 8, 512, 128, 128
HW = H * W          # 16384
K = 64              # kept DCT coefficients (8x8 band)
NCH = HW // 128     # 128 n-chunks of 128
NCI = HW // 512     # 32 n-chunks of 512
CCH = C // 128      # 4 c-chunks


def _getP():
    """DCT projection matrix P [HW, K], faithful to the reference."""
    Hs, Ws = H, W
    k = (0, 8, 0, 8)
    ind_h = 2.0 * np.arange(Hs) + 1.0
    Dht = np.stack(
        [np.sqrt(2.0) / np.sqrt(Hs) * np.cos(u * ind_h * np.pi / (2.0 * Hs)) for u in range(Hs)]
    ).astype(np.float32)
    Dht[0, :] = 1.0 / np.sqrt(Hs)
    Dh = Dht.T[:, k[0]:k[1]]
    ind_w = 2.0 * np.arange(Ws) + 1.0
    Dvt = np.stack(
        [np.sqrt(2.0) / np.sqrt(Hs) * np.cos(u * ind_w * np.pi / (2.0 * Ws)) for u in range(Ws)]
    ).astype(np.float32)
    Dvt[0, :] = 1.0 / np.sqrt(Ws)
    Dv = Dvt.T[:, k[2]:k[3]]
    P = np.einsum("hu,wv->hwuv", Dh, Dv).reshape(Hs * Ws, (k[1] - k[0]) * (k[3] - k[2]))
    return np.ascontiguousarray(P.astype(np.float32))


def _build():
    nc = bacc.Bacc("TRN2", target_bir_lowering=False, debug=False, enable_asserts=False)

    xT = nc.dram_tensor("xT", [HW, C], BF16, kind="ExternalInput")
    xn = nc.dram_tensor("xn", [C, HW], BF16, kind="ExternalInput")
    pext = nc.dram_tensor("pext", [128, NCH, K + 1], F32, kind="ExternalInput")
    pbf = nc.dram_tensor("pbf", [128, NCH, K], BF16, kind="ExternalInput")
    ptbf = nc.dram_tensor("ptbf", [K, HW], BF16, kind="ExternalInput")
    wcat = nc.dram_tensor("wcat", [128, CCH, 640], F32, kind="ExternalInput")
    ident = nc.dram_tensor("ident", [128, 128], F32, kind="ExternalInput")
    biasg = nc.dram_tensor("biasg", [1, C], F32, kind="ExternalInput")
    gam = nc.dram_tensor("gam", [1, 1], F32, kind="ExternalInput")
    srow = nc.dram_tensor("srow", [1, K], F32, kind="ExternalInput")
    out = nc.dram_tensor("out", [C, HW], F32, kind="ExternalOutput")

    with tile.TileContext(nc) as tc, ExitStack() as top:
        consts = top.enter_context(tc.tile_pool(name="consts", bufs=1))

        # ---- resident constants -------------------------------------------
        ptbf_sb = consts.tile([K, HW], BF16)
        nc.sync.dma_start(out=ptbf_sb, in_=ptbf[:, :])
        wcat_sb = consts.tile([128, CCH, 640], F32)
        nc.sync.dma_start(out=wcat_sb, in_=wcat[:, :, :])
        ident_sb = consts.tile([128, 128], F32)
        nc.sync.dma_start(out=ident_sb, in_=ident[:, :])
        bias_sb = consts.tile([1, C], F32)
        nc.sync.dma_start(out=bias_sb, in_=biasg[:, :])
        srow_sb = consts.tile([1, K], F32)
        nc.sync.dma_start(out=srow_sb, in_=srow[:, :])
        gamma_sb = consts.tile([128, 1], F32)
        nc.gpsimd.dma_start(out=gamma_sb, in_=gam[:, :].to_broadcast((128, 1)))

        ones_row = consts.tile([1, K], F32)
        nc.vector.memset(ones_row, 1.0)

        # ---- persistent intermediates -------------------------------------
        xpt_sb = consts.tile([K, C], F32)            # xP^T
        xp_sb = consts.tile([128, CCH, K], F32)      # xP chunks (c on partitions)
        qk_cat = consts.tile([K, 128], BF16)         # [WqxP^T | WkxP^T] bf16
        wqxp_sb = consts.tile([K, K], F32)
        wkxp_sb = consts.tile([K, K], F32)
        fatt_sb = consts.tile([K, K], F32)
        a_s_sb = consts.tile([K, K + 1], F32)        # [A | s]
        m1t_bf = consts.tile([K, K], BF16)
        rowv_bf = consts.tile([K, 1], BF16)
        wvg_bf = consts.tile([K + 1, C], BF16)       # [gamma*WvxP^T ; gamma*bias]
        lamq2 = consts.tile([128, NCH], F32)
        lamk2 = consts.tile([128, NCH], F32)
        rq_cols = consts.tile([128, NCH], F32)
        rlk_cols = consts.tile([128, NCH], F32)
        lpre_cols = consts.tile([128, NCH], F32)
        rv_cols = consts.tile([128, NCH], F32)
        rqv_cols = consts.tile([128, NCH], F32)

        # ---- stage 1: xPT = P^T @ x^T  ------------------------------------
        with tc.tile_pool(name="pbfp", bufs=1) as pbfp, \
             tc.tile_pool(name="s1psum", bufs=1, space="PSUM") as s1p, \
             tc.tile_pool(name="xtp", bufs=6) as xtp:
            pbf_sb = pbfp.tile([128, NCH, K], BF16)
            nc.sync.dma_start(out=pbf_sb, in_=pbf[:, :, :])
            ps_xpt = s1p.tile([K, C], F32)
            for h in range(NCH):
                xt_t = xtp.tile([128, C], BF16)
                nc.sync.dma_start(out=xt_t, in_=xT[h * 128:(h + 1) * 128, :])
                nc.tensor.matmul(
                    ps_xpt, lhsT=pbf_sb[:, h, :], rhs=xt_t,
                    start=(h == 0), stop=(h == NCH - 1),
                )
            nc.scalar.activation(out=xpt_sb, in_=ps_xpt, func=AF.Copy)

        # ---- stage 2: xP chunks via PE transpose --------------------------
        with tc.tile_pool(name="s2psum", bufs=2, space="PSUM") as s2p:
            for cc in range(CCH):
                ps_tr = s2p.tile([128, K], F32, tag="tr")
                nc.tensor.transpose(
                    ps_tr, xpt_sb[:, cc * 128:(cc + 1) * 128], ident_sb[0:K, 0:K]
                )
                nc.scalar.activation(out=xp_sb[:, cc, :], in_=ps_tr, func=AF.Copy)

        # ---- stage 3: W projections off xP --------------------------------
        with tc.tile_pool(name="s3psum", bufs=1, space="PSUM") as s3p:
            ps_w1 = s3p.tile([K, 512], F32, tag="w1")
            ps_w2 = s3p.tile([K, 128], F32, tag="w2")
            ps_q = s3p.tile([K, K], F32, tag="q")
            ps_k = s3p.tile([K, K], F32, tag="k")
            for cc in range(CCH):
                st, sp = (cc == 0), (cc == CCH - 1)
                nc.tensor.matmul(ps_w1, lhsT=xp_sb[:, cc, :], rhs=wcat_sb[:, cc, 0:512], start=st, stop=sp)
                nc.tensor.matmul(ps_w2, lhsT=xp_sb[:, cc, :], rhs=wcat_sb[:, cc, 512:640], start=st, stop=sp)
                nc.tensor.matmul(ps_q, lhsT=wcat_sb[:, cc, 0:64], rhs=xp_sb[:, cc, :], start=st, stop=sp)
                nc.tensor.matmul(ps_k, lhsT=wcat_sb[:, cc, 64:128], rhs=xp_sb[:, cc, :], start=st, stop=sp)
            # [WqxP^T | WkxP^T] -> bf16 stationaries for the norm streams
            nc.scalar.activation(out=qk_cat[:, 0:64], in_=ps_w1[:, 0:64], func=AF.Copy)
            nc.scalar.activation(out=qk_cat[:, 64:128], in_=ps_w1[:, 64:128], func=AF.Copy)
            # gamma * WvxP^T rows of the final stationary (bf16)
            nc.scalar.activation(out=wvg_bf[0:K, 0:384], in_=ps_w1[:, 128:512], func=AF.Copy,
                                 scale=gamma_sb[0:K, :])
            nc.scalar.activation(out=wvg_bf[0:K, 384:512], in_=ps_w2, func=AF.Copy,
                                 scale=gamma_sb[0:K, :])
            nc.scalar.activation(out=wvg_bf[K:K + 1, :], in_=bias_sb, func=AF.Copy,
                                 scale=gamma_sb[0:1, :])
            nc.scalar.activation(out=wqxp_sb, in_=ps_q, func=AF.Copy)
            nc.scalar.activation(out=wkxp_sb, in_=ps_k, func=AF.Copy)
            ps_f = s3p.tile([K, K], F32, tag="f")
            nc.tensor.matmul(ps_f, lhsT=wkxp_sb, rhs=wqxp_sb, start=True, stop=True)
            nc.scalar.activation(out=fatt_sb, in_=ps_f, func=AF.Copy)

        # ---- stages 4+5 under the pext scope ------------------------------
        with tc.tile_pool(name="pextp", bufs=1) as pextp:
            pext_sb = pextp.tile([128, NCH, K + 1], F32)
            nc.sync.dma_start(out=pext_sb, in_=pext[:, :, :])

            # ---- stage 4: QT/KT chunks + norm columns ---------------------
            with tc.tile_pool(name="s4psum", bufs=3, space="PSUM") as s4p, \
                 tc.tile_pool(name="s4sq", bufs=3) as s4sq:
                for ch in range(NCH):
                    ps_qk = s4p.tile([128, 128], F32, tag="qkt")
                    nc.tensor.matmul(
                        ps_qk, lhsT=ptbf_sb[:, ch * 128:(ch + 1) * 128], rhs=qk_cat,
                        start=True, stop=True,
                    )
                    sq = s4sq.tile([128, 128], F32, tag="sq")
                    nc.scalar.activation(out=sq[:, 0:64], in_=ps_qk[:, 0:64], func=AF.Square,
                                         accum_out=lamq2[:, ch:ch + 1])
                    nc.scalar.activation(out=sq[:, 64:128], in_=ps_qk[:, 64:128], func=AF.Square,
                                         accum_out=lamk2[:, ch:ch + 1])
                # rq = 1/sqrt(lamq2), rlk = 1/sqrt(lamk2)
                nc.scalar.activation(out=rq_cols, in_=lamq2, func=AF.Sqrt)
                nc.vector.reciprocal(rq_cols, rq_cols)
                nc.scalar.activation(out=rlk_cols, in_=lamk2, func=AF.Sqrt)
                nc.vector.reciprocal(rlk_cols, rlk_cols)

            # ---- stage 5: A_ext = Pk^T @ [P | 1] --------------------------
            with tc.tile_pool(name="s5psum", bufs=1, space="PSUM") as s5p, \
                 tc.tile_pool(name="s5pk", bufs=3) as s5pk:
                ps_a = s5p.tile([K, K + 1], F32)
                for ch in range(NCH):
                    pk = s5pk.tile([128, K], F32, tag="pk")
                    nc.scalar.activation(out=pk, in_=pext_sb[:, ch, 0:K], func=AF.Copy,
                                         scale=rlk_cols[:, ch:ch + 1])
                    nc.tensor.matmul(ps_a, lhsT=pk, rhs=pext_sb[:, ch, :],
                                     start=(ch == 0), stop=(ch == NCH - 1))
                nc.scalar.activation(out=a_s_sb, in_=ps_a, func=AF.Copy)

        # ---- stages 6+7 under the flats scope -----------------------------
        flp = top.enter_context(tc.tile_pool(name="flats", bufs=1))
        rv_flat = flp.tile([1, HW], F32)
        rqv_flat = flp.tile([1, HW], F32)

        # ---- stage 6: M1T/rowv, lamdv columns, rv/rqv ---------------------
        with tc.tile_pool(name="s6psum", bufs=2, space="PSUM") as s6p:
            ps_m = s6p.tile([K, K + 1], F32, tag="m")
            nc.tensor.matmul(ps_m, lhsT=fatt_sb, rhs=a_s_sb, start=True, stop=True)
            nc.scalar.activation(out=m1t_bf, in_=ps_m[:, 0:K], func=AF.Copy)
            nc.scalar.activation(out=rowv_bf, in_=ps_m[:, K:K + 1], func=AF.Copy)
            for ch in range(NCH):
                ps_lp = s6p.tile([128, 1], F32, tag="lp")
                nc.tensor.matmul(ps_lp, lhsT=ptbf_sb[:, ch * 128:(ch + 1) * 128],
                                 rhs=rowv_bf, start=True, stop=True)
                nc.scalar.activation(out=lpre_cols[:, ch:ch + 1], in_=ps_lp, func=AF.Copy)
            # rv = 1/(HW + lpre*rq) ; rqv = rq*rv
            nc.vector.tensor_mul(rv_cols, lpre_cols, rq_cols)
            nc.vector.tensor_scalar_add(rv_cols, rv_cols, float(HW))
            nc.vector.reciprocal(rv_cols, rv_cols)
            nc.vector.tensor_mul(rqv_cols, rv_cols, rq_cols)
            # flatten columns [128, NCH] into single-partition rows [1, HW]
            # flat index = p*NCH + ch (digit-swapped n; consumed swapped below)
            nc.gpsimd.dma_start(
                out=rv_flat[0:1, :].rearrange("o (p ch) -> o p ch", p=128),
                in_=rv_cols[:, :],
            )
            nc.gpsimd.dma_start(
                out=rqv_flat[0:1, :].rearrange("o (p ch) -> o p ch", p=128),
                in_=rqv_cols[:, :],
            )

        # swapped views: [o, ch, p] slices give n-ordered 512-wide rows
        rv_sw = rv_flat[0:1, :].rearrange("o (p ch) -> o ch p", p=128)
        rqv_sw = rqv_flat[0:1, :].rearrange("o (p ch) -> o ch p", p=128)

        # ---- stage 7: output chunks ---------------------------------------
        with tc.tile_pool(name="s7psum", bufs=2, space="PSUM") as s7p, \
             tc.tile_pool(name="s7t", bufs=3) as s7t, \
             tc.tile_pool(name="s7tmp", bufs=2) as s7tmp, \
             tc.tile_pool(name="s7xn", bufs=4) as s7xn, \
             tc.tile_pool(name="s7o", bufs=4) as s7o:
            for ci in range(NCI):
                nsl = slice(ci * 512, (ci + 1) * 512)
                ps_g = s7p.tile([K, 512], F32, tag="g")
                nc.tensor.matmul(ps_g, lhsT=m1t_bf, rhs=ptbf_sb[:, nsl],
                                 start=True, stop=True)
                ps_rqv = s7p.tile([K, 512], F32, tag="rqv")
                nc.tensor.matmul(ps_rqv, lhsT=ones_row,
                                 rhs=rqv_sw[:, 4 * ci:4 * ci + 4, :], start=True, stop=True)
                ps_srv = s7p.tile([K, 512], F32, tag="srv")
                nc.tensor.matmul(ps_srv, lhsT=srow_sb,
                                 rhs=rv_sw[:, 4 * ci:4 * ci + 4, :], start=True, stop=True)
                rqv_sb = s7tmp.tile([K, 512], F32, tag="rqvsb")
                nc.scalar.activation(out=rqv_sb, in_=ps_rqv, func=AF.Copy)
                tmp = s7tmp.tile([K, 512], F32, tag="tmp")
                nc.vector.tensor_mul(tmp, ps_g, rqv_sb)
                t_t = s7t.tile([K + 1, 512], BF16, tag="t")
                nc.vector.tensor_add(t_t[0:K, :], tmp, ps_srv)
                nc.vector.memset(t_t[K:K + 1, :], 1.0)
                for cs in range(CCH):
                    csl = slice(cs * 128, (cs + 1) * 128)
                    ps_o = s7p.tile([128, 512], F32, tag="o")
                    nc.tensor.matmul(ps_o, lhsT=wvg_bf[:, csl], rhs=t_t,
                                     start=True, stop=True)
                    xn_t = s7xn.tile([128, 512], BF16, tag="xn")
                    nc.sync.dma_start(out=xn_t, in_=xn[csl, nsl])
                    o_t = s7o.tile([128, 512], F32, tag="o")
                    nc.vector.tensor_add(o_t, ps_o, xn_t)
                    nc.sync.dma_start(out=out[csl, nsl], in_=o_t)

    nc.compile()
    return nc


_CACHE = {}


def _get_nc():
    if "nc" not in _CACHE:
        _CACHE["nc"] = _build()
    return _CACHE["nc"]


def _host_constants():
    if "consts" in _CACHE:
        return _CACHE["consts"]
    P = _getP()                                   # [HW, K] f32
    pext = np.ones((NCH, 128, K + 1), np.float32)
    pext[:, :, 0:K] = P.reshape(NCH, 128, K)
    pext = np.ascontiguousarray(pext.transpose(1, 0, 2))          # [p, ch, K+1]
    pbf = np.ascontiguousarray(
        P.reshape(NCH, 128, K).transpose(1, 0, 2).astype(BF16_NP))  # [p, h, K]
    ptbf = np.ascontiguousarray(P.T.astype(BF16_NP))              # [K, HW]
    srow = np.ascontiguousarray(P.sum(axis=0, dtype=np.float64).astype(np.float32)[None, :])
    ident = np.eye(128, dtype=np.float32)
    _CACHE["consts"] = (pext, pbf, ptbf, srow, ident)
    return _CACHE["consts"]


def kernel(x, Wq, Wk, Wv, out_bias, gamma):
    x = np.asarray(x, dtype=np.float32)
    Wq = np.asarray(Wq, dtype=np.float32)
    Wk = np.asarray(Wk, dtype=np.float32)
    Wv = np.asarray(Wv, dtype=np.float32)
    out_bias = np.asarray(out_bias, dtype=np.float32)
    gamma = np.asarray(gamma, dtype=np.float32)

    pext, pbf, ptbf, srow, ident = _host_constants()
    wcat_full = np.concatenate([Wq.T, Wk.T, Wv.T], axis=1)        # [C, 640]
    wcat = np.ascontiguousarray(wcat_full.reshape(CCH, 128, 640).transpose(1, 0, 2))
    biasg = np.ascontiguousarray(out_bias.reshape(1, C))
    gam = gamma.reshape(1, 1)

    x2 = x.reshape(B, C, HW)
    in_maps = []
    for b in range(B):
        xb = x2[b]
        in_maps.append({
            "xT": np.ascontiguousarray(xb.T).astype(BF16_NP),
            "xn": xb.astype(BF16_NP),
            "pext": pext, "pbf": pbf, "ptbf": ptbf, "wcat": wcat,
            "ident": ident, "biasg": biasg, "gam": gam, "srow": srow,
        })

    nc = _get_nc()
    res = run_bass_kernel_spmd(nc, in_maps, core_ids=list(range(B)))
    out = np.stack([res.results[b]["out"] for b in range(B)], axis=0)
    return out.reshape(B, C, H, W)


def bench(inputs, core_id=0, trace_cores=None):
    """Single-core traced run for timing (same SPMD program on every core)."""
    x = np.asarray(inputs["x"], dtype=np.float32)
    Wq = np.asarray(inputs["Wq"], dtype=np.float32)
    Wk = np.asarray(inputs["Wk"], dtype=np.float32)
    Wv = np.asarray(inputs["Wv"], dtype=np.float32)
    out_bias = np.asarray(inputs["out_bias"], dtype=np.float32)
    gamma = np.asarray(inputs["gamma"], dtype=np.float32)

    pext, pbf, ptbf, srow, ident = _host_constants()
    wcat_full = np.concatenate([Wq.T, Wk.T, Wv.T], axis=1)
    wcat = np.ascontiguousarray(wcat_full.reshape(CCH, 128, 640).transpose(1, 0, 2))
    biasg = np.ascontiguousarray(out_bias.reshape(1, C))
    gam = gamma.reshape(1, 1)
    xb = x.reshape(B, C, HW)[0]
    in_map = {
        "xT": np.ascontiguousarray(xb.T).astype(BF16_NP),
        "xn": xb.astype(BF16_NP),
        "pext": pext, "pbf": pbf, "ptbf": ptbf, "wcat": wcat,
        "ident": ident, "biasg": biasg, "gam": gam, "srow": srow,
    }
    nc = _get_nc()
    res = run_bass_kernel_spmd(nc, [in_map], core_ids=[core_id], trace=True)
    return res.exec_time_ns


if __name__ == "__main__":
    rng = np.random.default_rng(0)
    x = rng.standard_normal((B, C, H, W), dtype=np.float32)
    Wq = (rng.standard_normal((K, C)) * 0.05).astype(np.float32)
    Wk = (rng.standard_normal((K, C)) * 0.05).astype(np.float32)
    Wv = (rng.standard_normal((C, C)) * 0.05).astype(np.float32)
    ob = (rng.standard_normal((1, C, 1)) * 0.01).astype(np.float32)
    g = (rng.standard_normal((1,)) * 0.5).astype(np.float32)
    y = kernel(x=x, Wq=Wq, Wk=Wk, Wv=Wv, out_bias=ob, gamma=g)
    print("out", y.shape, y.dtype, float(np.abs(y).mean()))


# revision 19
# speedup vs baseline: 1.1024x; 1.1024x over previous
"""DCT non-local attention (nn_DCTNLAttention11) Trainium2 kernel.

Data-parallel over batch B=8 across 8 NeuronCores; each core processes one
batch element [C=512, HW=16384].  All constants derived from the DCT basis P
are precomputed on host; the per-core device program is:

  1. xPT = P^T @ x^T            [64, 512]   (128 accumulated matmuls over n)
  2. xP (PE transposes), then W-projections off xP (tiny matmuls):
     WqxP^T/WkxP^T/WvxP^T, WqxP/WkxP, fatt = WkxP^T @ WqxP
  3. Per-n norm columns: QT/KT chunks [128,128] via PT-chunk-stationary
     matmuls; lamq2/lamk2 via DVE tensor_tensor_reduce; 1/sqrt batched.
  4. Pk = P * (1/lamdk) per chunk (GpSimd); A_ext = Pk^T @ [P|1] (accumulated).
  5. M1T/rowv = fatt^T @ [A|s]; lamdv_pre columns (batched [128,16] psums);
     rv/rqv columns; lamdq/rqv flattened to [1, HW] rows via DMA.
  6. Per 512-col chunk n (pairs share stationaries):
       U   = M1 @ PT + S (x) lamdq        (PSUM accumulation, two matmuls)
       T   = U * rqv_bcast (rank-1 matmul + DVE, identity
             G*rq*rv + S*rv == (G + S (x) lamdq)*rqv)
       out = [gamma*WvxP^T; gamma*bias]^T @ [T; 1] + x    (residual bf16)
"""

import numpy as np
import ml_dtypes
from contextlib import ExitStack

import concourse.bass as bass
import concourse.bacc as bacc
import concourse.tile as tile
from concourse import mybir
from concourse.bass_utils import run_bass_kernel_spmd

F32 = mybir.dt.float32
BF16 = mybir.dt.bfloat16
AF = mybir.ActivationFunctionType
ALU = mybir.AluOpType
BF16_NP = ml_dtypes.bfloat16

B, C, H, W = 8, 512, 128, 128
HW = H * W          # 16384
K = 64              # kept DCT coefficients (8x8 band)
NCH = HW // 128     # 128 n-chunks of 128
NCI = HW // 512     # 32 n-chunks of 512
CCH = C // 128      # 4 c-chunks


def _getP():
    """DCT projection matrix P [HW, K], faithful to the reference."""
    Hs, Ws = H, W
    k = (0, 8, 0, 8)
    ind_h = 2.0 * np.arange(Hs) + 1.0
    Dht = np.stack(
        [np.sqrt(2.0) / np.sqrt(Hs) * np.cos(u * ind_h * np.pi / (2.0 * Hs)) for u in range(Hs)]
    ).astype(np.float32)
    Dht[0, :] = 1.0 / np.sqrt(Hs)
    Dh = Dht.T[:, k[0]:k[1]]
    ind_w = 2.0 * np.arange(Ws) + 1.0
    Dvt = np.stack(
        [np.sqrt(2.0) / np.sqrt(Hs) * np.cos(u * ind_w * np.pi / (2.0 * Ws)) for u in range(Ws)]
    ).astype(np.float32)
    Dvt[0, :] = 1.0 / np.sqrt(Ws)
    Dv = Dvt.T[:, k[2]:k[3]]
    P = np.einsum("hu,wv->hwuv", Dh, Dv).reshape(Hs * Ws, (k[1] - k[0]) * (k[3] - k[2]))
    return np.ascontiguousarray(P.astype(np.float32))


def _build():
    nc = bacc.Bacc("TRN2", target_bir_lowering=False, debug=False, enable_asserts=False)

    xT = nc.dram_tensor("xT", [HW, C], BF16, kind="ExternalInput")
    xn = nc.dram_tensor("xn", [C, HW], BF16, kind="ExternalInput")
    pext = nc.dram_tensor("pext", [128, NCH, K + 1], F32, kind="ExternalInput")
    pbf = nc.dram_tensor("pbf", [128, NCH, K], BF16, kind="ExternalInput")
    ptbf = nc.dram_tensor("ptbf", [K, HW], BF16, kind="ExternalInput")
    wcat = nc.dram_tensor("wcat", [128, CCH, 640], F32, kind="ExternalInput")
    ident = nc.dram_tensor("ident", [128, 128], F32, kind="ExternalInput")
    biasg = nc.dram_tensor("biasg", [1, C], F32, kind="ExternalInput")
    gam = nc.dram_tensor("gam", [1, 1], F32, kind="ExternalInput")
    srow = nc.dram_tensor("srow", [1, K], F32, kind="ExternalInput")
    out = nc.dram_tensor("out", [C, HW], F32, kind="ExternalOutput")
    flb = nc.dram_tensor("flbounce", [2, 128, NCH], F32, kind="Internal")

    with tile.TileContext(nc) as tc, ExitStack() as top:
        consts = top.enter_context(tc.tile_pool(name="consts", bufs=1))

        # persistent intermediates / small constants
        ptbf_sb = consts.tile([K, HW], BF16)
        ident_sb = consts.tile([128, 128], F32)
        bias_sb = consts.tile([1, C], F32)
        srow_sb = consts.tile([1, K], F32)
        gamma_sb = consts.tile([128, 1], F32)
        ones_row = consts.tile([1, K], F32)
        xpt_sb = consts.tile([K, C], F32)            # xP^T
        xp_sb = consts.tile([128, CCH, K], F32)      # xP chunks (c on partitions)
        qk_cat = consts.tile([K, 128], BF16)         # [WqxP^T | WkxP^T] bf16
        wqxp_sb = consts.tile([K, K], F32)
        wkxp_sb = consts.tile([K, K], F32)
        fatt_sb = consts.tile([K, K], F32)
        a_s_sb = consts.tile([K, K + 1], F32)        # [A | s]
        m1t_bf = consts.tile([K, K], BF16)
        rowv_bf = consts.tile([K, 1], BF16)
        wvg_bf = consts.tile([K + 1, C], BF16)       # [gamma*WvxP^T ; gamma*bias]
        lamq_mv = consts.tile([128, NCH, 2], F32)    # bn_aggr (mean, var) per chunk
        lamk_mv = consts.tile([128, NCH, 2], F32)
        lamdq_cols = consts.tile([128, NCH], F32)
        rq_cols = consts.tile([128, NCH], F32)
        rlk_cols = consts.tile([128, NCH], F32)
        lpre_cols = consts.tile([128, NCH], F32)
        rv_cols = consts.tile([128, NCH], F32)
        rqv_cols = consts.tile([128, NCH], F32)

        # ---- stage 1: xPT = P^T @ x^T  ------------------------------------
        # critical path first: pbf halves then xT tiles on the Sync ring;
        # consts go on the Scalar HWDGE ring so they don't head-of-line block.
        with tc.tile_pool(name="pbfp", bufs=1) as pbfp, \
             tc.tile_pool(name="s1psum", bufs=1, space="PSUM") as s1p, \
             tc.tile_pool(name="xtp", bufs=6) as xtp:
            pbf_sb = pbfp.tile([128, NCH, K], BF16)
            nc.sync.dma_start(out=pbf_sb[:, 0:32, :], in_=pbf[:, 0:32, :])
            nc.sync.dma_start(out=pbf_sb[:, 32:NCH, :], in_=pbf[:, 32:NCH, :])
            nc.scalar.dma_start(out=ptbf_sb, in_=ptbf[:, :])
            nc.scalar.dma_start(out=ident_sb, in_=ident[:, :])
            nc.scalar.dma_start(out=bias_sb, in_=biasg[:, :])
            nc.scalar.dma_start(out=srow_sb, in_=srow[:, :])
            nc.gpsimd.dma_start(out=gamma_sb, in_=gam[:, :].to_broadcast((128, 1)))
            nc.vector.memset(ones_row, 1.0)

            ps_xpt = s1p.tile([K, C], F32)
            xT2 = xT[:, :].rearrange("(h2 j p) c -> h2 p j c", j=2, p=128)
            for h2 in range(NCH // 2):
                xt_t = xtp.tile([128, 2, C], BF16)
                nc.sync.dma_start(out=xt_t, in_=xT2[h2])
                for j in range(2):
                    h = 2 * h2 + j
                    nc.tensor.matmul(
                        ps_xpt, lhsT=pbf_sb[:, h, :], rhs=xt_t[:, j, :],
                        start=(h == 0), stop=(h == NCH - 1),
                    )
            nc.scalar.activation(out=xpt_sb, in_=ps_xpt, func=AF.Copy)

        # ---- stage 2+3: xP via PE transpose; W projections ----------------
        with tc.tile_pool(name="wcatp", bufs=1) as wcatp, \
             tc.tile_pool(name="s2psum", bufs=2, space="PSUM") as s2p, \
             tc.tile_pool(name="s3psum", bufs=1, space="PSUM") as s3p:
            wcat_sb = wcatp.tile([128, CCH, 640], F32)
            nc.scalar.dma_start(out=wcat_sb, in_=wcat[:, :, :])
            for cc in range(CCH):
                ps_tr = s2p.tile([128, K], F32, tag="tr")
                nc.tensor.transpose(
                    ps_tr, xpt_sb[:, cc * 128:(cc + 1) * 128], ident_sb[0:K, 0:K]
                )
                nc.scalar.activation(out=xp_sb[:, cc, :], in_=ps_tr, func=AF.Copy)

            ps_w1 = s3p.tile([K, 512], F32, tag="w1")
            ps_w2 = s3p.tile([K, 128], F32, tag="w2")
            ps_q = s3p.tile([K, K], F32, tag="q")
            ps_k = s3p.tile([K, K], F32, tag="k")
            for cc in range(CCH):
                st, sp = (cc == 0), (cc == CCH - 1)
                nc.tensor.matmul(ps_w1, lhsT=xp_sb[:, cc, :], rhs=wcat_sb[:, cc, 0:512], start=st, stop=sp)
                nc.tensor.matmul(ps_w2, lhsT=xp_sb[:, cc, :], rhs=wcat_sb[:, cc, 512:640], start=st, stop=sp)
                nc.tensor.matmul(ps_q, lhsT=wcat_sb[:, cc, 0:64], rhs=xp_sb[:, cc, :], start=st, stop=sp)
                nc.tensor.matmul(ps_k, lhsT=wcat_sb[:, cc, 64:128], rhs=xp_sb[:, cc, :], start=st, stop=sp)
            nc.scalar.activation(out=qk_cat[:, 0:64], in_=ps_w1[:, 0:64], func=AF.Copy)
            nc.scalar.activation(out=qk_cat[:, 64:128], in_=ps_w1[:, 64:128], func=AF.Copy)
            nc.scalar.activation(out=wvg_bf[0:K, 0:384], in_=ps_w1[:, 128:512], func=AF.Copy,
                                 scale=gamma_sb[0:K, :])
            nc.scalar.activation(out=wvg_bf[0:K, 384:512], in_=ps_w2, func=AF.Copy,
                                 scale=gamma_sb[0:K, :])
            nc.scalar.activation(out=wvg_bf[K:K + 1, :], in_=bias_sb, func=AF.Copy,
                                 scale=gamma_sb[0:1, :])
            nc.scalar.activation(out=wqxp_sb, in_=ps_q, func=AF.Copy)
            nc.scalar.activation(out=wkxp_sb, in_=ps_k, func=AF.Copy)
            ps_f = s3p.tile([K, K], F32, tag="f")
            nc.tensor.matmul(ps_f, lhsT=wkxp_sb, rhs=wqxp_sb, start=True, stop=True)
            nc.scalar.activation(out=fatt_sb, in_=ps_f, func=AF.Copy)

        # ---- stages 4+5 under the pext scope ------------------------------
        with tc.tile_pool(name="pextp", bufs=1) as pextp:
            pext_sb = pextp.tile([128, NCH, K + 1], F32)
            nc.scalar.dma_start(out=pext_sb[:, 0:64, :], in_=pext[:, 0:64, :])
            nc.scalar.dma_start(out=pext_sb[:, 64:NCH, :], in_=pext[:, 64:NCH, :])

            # ---- stage 4: QT/KT chunks + norm columns ---------------------
            with tc.tile_pool(name="s4psum", bufs=3, space="PSUM") as s4p, \
                 tc.tile_pool(name="s4sq", bufs=3) as s4sq:
                for ch in range(NCH):
                    ps_qk = s4p.tile([128, 128], F32, tag="qkt")
                    nc.tensor.matmul(
                        ps_qk, lhsT=ptbf_sb[:, ch * 128:(ch + 1) * 128], rhs=qk_cat,
                        start=True, stop=True,
                    )
                    sq = s4sq.tile([128, 128], F32, tag="sq")
                    nc.scalar.activation(out=sq, in_=ps_qk, func=AF.Square)
                    st = s4sq.tile([128, 2, 6], F32, tag="st")
                    nc.vector.bn_stats(out=st[:, 0, :], in_=sq[:, 0:64])
                    nc.vector.bn_aggr(out=lamq_mv[:, ch, :], in_=st[:, 0, :])
                    nc.vector.bn_stats(out=st[:, 1, :], in_=sq[:, 64:128])
                    nc.vector.bn_aggr(out=lamk_mv[:, ch, :], in_=st[:, 1, :])
                # lamdq = sqrt(64*mean_q); rq = 1/lamdq; rlk = 1/sqrt(64*mean_k)
                nc.scalar.activation(out=lamdq_cols, in_=lamq_mv[:, :, 0], func=AF.Sqrt,
                                     scale=64.0)
                nc.vector.reciprocal(rq_cols, lamdq_cols)
                nc.scalar.activation(out=rlk_cols, in_=lamk_mv[:, :, 0], func=AF.Sqrt,
                                     scale=64.0)
                nc.vector.reciprocal(rlk_cols, rlk_cols)

            # ---- stage 5: A_ext = Pk^T @ [P | 1] --------------------------
            with tc.tile_pool(name="s5psum", bufs=1, space="PSUM") as s5p, \
                 tc.tile_pool(name="s5pk", bufs=3) as s5pk:
                ps_a = s5p.tile([K, K + 1], F32)
                for ch in range(NCH):
                    pk = s5pk.tile([128, K], F32, tag="pk")
                    nc.gpsimd.tensor_scalar_mul(pk, pext_sb[:, ch, 0:K],
                                                rlk_cols[:, ch:ch + 1])
                    nc.tensor.matmul(ps_a, lhsT=pk, rhs=pext_sb[:, ch, :],
                                     start=(ch == 0), stop=(ch == NCH - 1))
                nc.scalar.activation(out=a_s_sb, in_=ps_a, func=AF.Copy)

        # ---- stages 6+7 under the flats scope -----------------------------
        flp = top.enter_context(tc.tile_pool(name="flats", bufs=1))
        lamdq_flat = flp.tile([1, HW], F32)
        rqv_flat = flp.tile([1, HW], F32)

        # ---- stage 6: M1T/rowv, lamdv columns, rv/rqv ---------------------
        with tc.tile_pool(name="s6psum", bufs=2, space="PSUM") as s6p:
            ps_m = s6p.tile([K, K + 1], F32, tag="m")
            nc.tensor.matmul(ps_m, lhsT=fatt_sb, rhs=a_s_sb, start=True, stop=True)
            nc.scalar.activation(out=m1t_bf, in_=ps_m[:, 0:K], func=AF.Copy)
            nc.scalar.activation(out=rowv_bf, in_=ps_m[:, K:K + 1], func=AF.Copy)
            for g in range(NCH // 16):
                ps_lp = s6p.tile([128, 16], F32, tag="lp")
                for j in range(16):
                    ch = g * 16 + j
                    nc.tensor.matmul(ps_lp[:, j:j + 1],
                                     lhsT=ptbf_sb[:, ch * 128:(ch + 1) * 128],
                                     rhs=rowv_bf, start=True, stop=True)
                nc.scalar.activation(out=lpre_cols[:, g * 16:(g + 1) * 16],
                                     in_=ps_lp, func=AF.Copy)
            # rv = 1/(HW + lpre*rq) ; rqv = rq*rv
            nc.vector.tensor_mul(rv_cols, lpre_cols, rq_cols)
            nc.vector.tensor_scalar_add(rv_cols, rv_cols, float(HW))
            nc.vector.reciprocal(rv_cols, rv_cols)
            nc.vector.tensor_mul(rqv_cols, rv_cols, rq_cols)
            # flatten columns [128, NCH] into single-partition rows [1, HW] via
            # a DRAM bounce; flat index = p*NCH + ch (digit-swapped n, consumed
            # swapped below)
            nc.sync.dma_start(out=flb[0, :, :], in_=lamdq_cols[:, :])
            nc.sync.dma_start(out=flb[1, :, :], in_=rqv_cols[:, :])
            nc.sync.dma_start(
                out=lamdq_flat[0:1, :].rearrange("o (p ch) -> o p ch", p=128),
                in_=flb[0:1, :, :],
            )
            nc.sync.dma_start(
                out=rqv_flat[0:1, :].rearrange("o (p ch) -> o p ch", p=128),
                in_=flb[1:2, :, :],
            )

        # swapped views: [o, ch, p] slices give n-ordered 512-wide rows
        ldq_sw = lamdq_flat[0:1, :].rearrange("o (p ch) -> o ch p", p=128)
        rqv_sw = rqv_flat[0:1, :].rearrange("o (p ch) -> o ch p", p=128)

        # ---- stage 7: output chunks (pairs share stationaries) ------------
        with tc.tile_pool(name="s7psum", bufs=1, space="PSUM") as s7p, \
             tc.tile_pool(name="s7psumo", bufs=2, space="PSUM") as s7po, \
             tc.tile_pool(name="s7t", bufs=4) as s7t, \
             tc.tile_pool(name="s7tmp", bufs=1) as s7tmp, \
             tc.tile_pool(name="s7xn", bufs=5) as s7xn, \
             tc.tile_pool(name="s7o", bufs=3) as s7o:
            xn2 = xn[:, :].rearrange("c (i2 j n) -> i2 c j n", j=2, n=512)
            out2 = out[:, :].rearrange("c (i2 j n) -> i2 c j n", j=2, n=512)
            for i2 in range(NCI // 2):
                ps_u = []
                for j in range(2):
                    ci = 2 * i2 + j
                    nsl = slice(ci * 512, (ci + 1) * 512)
                    pu = s7p.tile([K, 512], F32, tag=f"u{j}")
                    # U = M1 @ PT + S (x) lamdq   (accumulated in PSUM)
                    nc.tensor.matmul(pu, lhsT=m1t_bf, rhs=ptbf_sb[:, nsl],
                                     start=True, stop=False)
                    nc.tensor.matmul(pu, lhsT=srow_sb,
                                     rhs=ldq_sw[:, 4 * ci:4 * ci + 4, :],
                                     start=False, stop=True)
                    ps_u.append(pu)
                t_ts = []
                for j in range(2):
                    ci = 2 * i2 + j
                    ps_rqv = s7p.tile([K, 512], F32, tag=f"rqv{j}")
                    nc.tensor.matmul(ps_rqv, lhsT=ones_row,
                                     rhs=rqv_sw[:, 4 * ci:4 * ci + 4, :],
                                     start=True, stop=True)
                    rqv_sb = s7tmp.tile([K, 512], F32, tag=f"rqvsb{j}")
                    nc.scalar.activation(out=rqv_sb, in_=ps_rqv, func=AF.Copy)
                    t_t = s7t.tile([K + 1, 512], BF16, tag=f"t{j}")
                    nc.vector.tensor_mul(t_t[0:K, :], ps_u[j], rqv_sb)
                    nc.vector.memset(t_t[K:K + 1, :], 1.0)
                    t_ts.append(t_t)
                xn_t = []
                o_t = []
                for cs in range(CCH):
                    xt = s7xn.tile([128, 2, 512], BF16, tag="xn")
                    nc.sync.dma_start(out=xt, in_=xn2[i2, cs * 128:(cs + 1) * 128])
                    ot = s7o.tile([128, 2, 512], F32, tag="o")
                    for j in range(2):
                        ps_o = s7po.tile([128, 512], F32, tag=f"o{j}")
                        nc.tensor.matmul(ps_o, lhsT=wvg_bf[:, cs * 128:(cs + 1) * 128],
                                         rhs=t_ts[j], start=True, stop=True)
                        nc.vector.tensor_add(ot[:, j, :], ps_o, xt[:, j, :])
                    nc.scalar.dma_start(out=out2[i2, cs * 128:(cs + 1) * 128], in_=ot)

    nc.compile()
    return nc


_CACHE = {}


def _get_nc():
    if "nc" not in _CACHE:
        _CACHE["nc"] = _build()
    return _CACHE["nc"]


def _host_constants():
    if "consts" in _CACHE:
        return _CACHE["consts"]
    P = _getP()                                   # [HW, K] f32
    pext = np.ones((NCH, 128, K + 1), np.float32)
    pext[:, :, 0:K] = P.reshape(NCH, 128, K)
    pext = np.ascontiguousarray(pext.transpose(1, 0, 2))          # [p, ch, K+1]
    pbf = np.ascontiguousarray(
        P.reshape(NCH, 128, K).transpose(1, 0, 2).astype(BF16_NP))  # [p, h, K]
    ptbf = np.ascontiguousarray(P.T.astype(BF16_NP))              # [K, HW]
    srow = np.ascontiguousarray(P.sum(axis=0, dtype=np.float64).astype(np.float32)[None, :])
    ident = np.eye(128, dtype=np.float32)
    _CACHE["consts"] = (pext, pbf, ptbf, srow, ident)
    return _CACHE["consts"]


def _make_in_map(xb, Wq, Wk, Wv, out_bias, gamma):
    pext, pbf, ptbf, srow, ident = _host_constants()
    wcat_full = np.concatenate([Wq.T, Wk.T, Wv.T], axis=1)        # [C, 640]
    wcat = np.ascontiguousarray(wcat_full.reshape(CCH, 128, 640).transpose(1, 0, 2))
    biasg = np.ascontiguousarray(out_bias.reshape(1, C))
    gam = gamma.reshape(1, 1)
    return {
        "xT": np.ascontiguousarray(xb.T).astype(BF16_NP),
        "xn": xb.astype(BF16_NP),
        "pext": pext, "pbf": pbf, "ptbf": ptbf, "wcat": wcat,
        "ident": ident, "biasg": biasg, "gam": gam, "srow": srow,
    }


def kernel(x, Wq, Wk, Wv, out_bias, gamma):
    x = np.asarray(x, dtype=np.float32)
    Wq = np.asarray(Wq, dtype=np.float32)
    Wk = np.asarray(Wk, dtype=np.float32)
    Wv = np.asarray(Wv, dtype=np.float32)
    out_bias = np.asarray(out_bias, dtype=np.float32)
    gamma = np.asarray(gamma, dtype=np.float32)

    x2 = x.reshape(B, C, HW)
    in_maps = [_make_in_map(x2[b], Wq, Wk, Wv, out_bias, gamma) for b in range(B)]

    nc = _get_nc()
    res = run_bass_kernel_spmd(nc, in_maps, core_ids=list(range(B)))
    out = np.stack([res.results[b]["out"] for b in range(B)], axis=0)
    return out.reshape(B, C, H, W)


def bench(inputs, core_id=0):
    """Single-core traced run for timing (same SPMD program on every core)."""
    x = np.asarray(inputs["x"], dtype=np.float32)
    xb = x.reshape(B, C, HW)[0]
    in_map = _make_in_map(
        xb,
        np.asarray(inputs["Wq"], dtype=np.float32),
        np.asarray(inputs["Wk"], dtype=np.float32),
        np.asarray(inputs["Wv"], dtype=np.float32),
        np.asarray(inputs["out_bias"], dtype=np.float32),
        np.asarray(inputs["gamma"], dtype=np.float32),
    )
    nc = _get_nc()
    res = run_bass_kernel_spmd(nc, [in_map], core_ids=[core_id], trace=True)
    return res.exec_time_ns


if __name__ == "__main__":
    rng = np.random.default_rng(0)
    x = rng.standard_normal((B, C, H, W), dtype=np.float32)
    Wq = (rng.standard_normal((K, C)) * 0.05).astype(np.float32)
    Wk = (rng.standard_normal((K, C)) * 0.05).astype(np.float32)
    Wv = (rng.standard_normal((C, C)) * 0.05).astype(np.float32)
    ob = (rng.standard_normal((1, C, 1)) * 0.01).astype(np.float32)
    g = (rng.standard_normal((1,)) * 0.5).astype(np.float32)
    y = kernel(x=x, Wq=Wq, Wk=Wk, Wv=Wv, out_bias=ob, gamma=g)
    print("out", y.shape, y.dtype, float(np.abs(y).mean()))
